# revision 2
# baseline (speedup 1.0000x reference)
"""Distributed Trainium2 kernel for pre-LN causal multi-head attention.

Problem: out = x + Wo-proj(causal-MHA(LN(x))) with B=4, S=2048, D=1024,
H=16 heads, d_k=d_v=64, fp32.

Sharding over 8 NeuronCores (per the TP/DP hint):
  core r -> batch b = r//2, head group g = r%2 (heads 8g..8g+7).
  Wq/Wk/Wv column-sliced per head group, Wo row-sliced; the two cores of a
  pair {2p, 2p+1} each compute a partial output projection for batch p and
  a pairwise ReduceScatter (+ pre-added x/2 residual on each core) yields
  final output rows split across the pair.

Single-core layout strategy (everything transposed once, then no more
transposes):
  - LN stats in natural layout (bn_stats), gamma/beta folded into the
    projection weights host-side, so the device only standardizes.
  - xn^T via PE transposes; Q/K projections then produce q^T/k^T
    ([feature, token], features = head pairs stacked 64+64 on partitions)
    and V is produced in natural [token, feature] layout directly.
  - scores are computed TRANSPOSED: s^T[k, q] = k^T(stationary) x q^T(moving),
    two heads concurrently via PE row groups (base partitions 0 / 64).
  - softmax over k = partition axis of s^T: exp on ACT (no max subtraction
    needed: |scores/8| < ~3 by construction), denominator via a ones column
    appended to V in the P^T @ V matmul, causal mask via a precomputed
    sliding band mask multiply on the diagonal tiles only (plus skipping
    fully-masked tiles).
  - attn^T [d_v, q] feeds the output projection as stationary operand,
    producing y in natural [token, d_model] layout; +x/2 residual, then
    chunked (512-token) pairwise ReduceScatter overlapped with compute.
"""

import numpy as np

import concourse.bass as bass
import concourse.tile as tile
from concourse import bacc, mybir
from concourse.bass import ds, ts
from concourse.bass_utils import run_bass_kernel_spmd
from concourse.masks import make_identity

F32 = mybir.dt.float32
AF = mybir.ActivationFunctionType

B = 4
S = 2048
D = 1024
H = 16
DK = 64
H_LOC = 8            # heads per core
F_LOC = H_LOC * DK   # 512 local features
SCH = 512            # token chunk (pipeline + RS granularity)
NCH = S // SCH       # 4 chunks
NTT = SCH // 128     # 4 token tiles per chunk
NDC = D // 128       # 8 d_model chunks
NPC = F_LOC // 128   # 4 feature pair-chunks (2 heads each)
NKT = S // 128       # 16 key tiles
EPS = 1e-5
RG = [[0, 1], [2, 3], [4, 5], [6, 7]]


def build(n_chunks: int = NCH):
    """Build the SPMD graph (identical on all 8 cores)."""
    nc = bacc.Bacc("TRN2", target_bir_lowering=False, debug=False, num_devices=8)

    s_loc = n_chunks * SCH
    x_ext = nc.dram_tensor("x", [s_loc, D], F32, kind="ExternalInput").ap()
    wq_ext = nc.dram_tensor("wq", [D, F_LOC], F32, kind="ExternalInput").ap()
    wk_ext = nc.dram_tensor("wk", [D, F_LOC], F32, kind="ExternalInput").ap()
    wv_ext = nc.dram_tensor("wv", [D, F_LOC], F32, kind="ExternalInput").ap()
    wo_ext = nc.dram_tensor("wo", [F_LOC, D], F32, kind="ExternalInput").ap()
    mask_ext = nc.dram_tensor("mask", [128, 896], F32, kind="ExternalInput").ap()
    out_ext = nc.dram_tensor("out", [s_loc // 2, D], F32, kind="ExternalOutput").ap()

    with tile.TileContext(nc) as tc:
        with (
            tc.tile_pool(name="persist", bufs=1) as persist,
            tc.tile_pool(name="slabs", bufs=1) as slabs,
            tc.tile_pool(name="xp", bufs=3) as xp,
            tc.tile_pool(name="ptp", bufs=3) as ptp,
            tc.tile_pool(name="dnp", bufs=1) as dnp,
            tc.tile_pool(name="stp", bufs=3) as stp,
            tc.tile_pool(name="ps_tr", bufs=2, space="PSUM") as ps_tr,
            tc.tile_pool(name="ps_big", bufs=2, space="PSUM") as ps_big,
            tc.tile_pool(name="ps_sc", bufs=2, space="PSUM") as ps_sc,
            tc.tile_pool(name="ps_out", bufs=2, space="PSUM") as ps_out,
            tc.tile_pool(name="dram", bufs=2, space="DRAM") as dram,
        ):
            # ---- persistent tiles ----
            wq_sb = persist.tile([128, NDC, F_LOC], F32)
            wk_sb = persist.tile([128, NDC, F_LOC], F32)
            wv_sb = persist.tile([128, NDC, F_LOC], F32)
            wo_sb = persist.tile([128, NPC, D], F32)
            for dc in range(NDC):
                nc.sync.dma_start(out=wq_sb[:, dc, :], in_=wq_ext[ds(dc * 128, 128), :])
                nc.sync.dma_start(out=wk_sb[:, dc, :], in_=wk_ext[ds(dc * 128, 128), :])
                nc.sync.dma_start(out=wv_sb[:, dc, :], in_=wv_ext[ds(dc * 128, 128), :])
            for pc in range(NPC):
                nc.sync.dma_start(out=wo_sb[:, pc, :], in_=wo_ext[ds(pc * 128, 128), :])

            mask_sb = persist.tile([128, 896], F32)
            nc.sync.dma_start(out=mask_sb[:], in_=mask_ext[:])
            ident = persist.tile([128, 128], F32)
            make_identity(nc, ident)
            ones64 = persist.tile([1, 64], F32)
            nc.vector.memset(ones64, 1.0)
            epsb = persist.tile([128, 1], F32)
            nc.vector.memset(epsb, EPS)

            # k^T per head pair: [128 (= 2x64 head dims), S]
            kT = [persist.tile([128, S], F32, name=f"kT{p}") for p in range(NPC)]
            # v (+ ones col per head) per key tile: [128 tokens, 8*(64+1)]
            vsb = [persist.tile([128, H_LOC * 65], F32, name=f"v{t}") for t in range(NKT)]
            for t in range(n_chunks * NTT):
                v3 = vsb[t].rearrange("p (h c) -> p h c", h=H_LOC)
                nc.vector.memset(v3[:, :, 64:65], 1.0)

            for j in range(n_chunks):
                # ---------- LayerNorm + transpose ----------
                xnT = slabs.tile([128, NDC, SCH], F32, tag="xnT")
                for tt in range(NTT):
                    g = j * NTT + tt
                    x_t = xp.tile([128, D], F32, tag="x_t")
                    nc.sync.dma_start(out=x_t[:], in_=x_ext[ds(g * 128, 128), :])
                    st6 = stp.tile([128, 2, 6], F32)
                    nc.vector.bn_stats(st6[:, 0, :], x_t[:, 0:512])
                    nc.vector.bn_stats(st6[:, 1, :], x_t[:, 512:1024])
                    mv = stp.tile([128, 2], F32)
                    nc.vector.bn_aggr(mv, st6)
                    rstd = stp.tile([128, 1], F32)
                    nc.scalar.activation(rstd, mv[:, 1:2], AF.Sqrt, bias=epsb)
                    nc.vector.reciprocal(rstd, rstd)
                    nc.vector.tensor_scalar(
                        out=x_t[:],
                        in0=x_t[:],
                        scalar1=mv[:, 0:1],
                        scalar2=rstd,
                        op0=mybir.AluOpType.subtract,
                        op1=mybir.AluOpType.mult,
                    )
                    for half in range(2):
                        ptr = ps_tr.tile([128, 512], F32, tag="tr")
                        for q in range(4):
                            dc = half * 4 + q
                            nc.tensor.transpose(
                                ptr[:, ts(q, 128)], x_t[:, ts(dc, 128)], ident
                            )
                        nc.any.tensor_copy(
                            xnT[:, ds(half * 4, 4), ts(tt, 128)],
                            ptr.rearrange("p (c n) -> p c n", c=4),
                        )

                # ---------- Q/K/V projections for this chunk ----------
                qT = slabs.tile([128, NPC, SCH], F32, tag="qT")
                for pc in range(NPC):
                    psq = ps_big.tile([128, SCH], F32, tag="big")
                    for dc in range(NDC):
                        nc.tensor.matmul(
                            psq,
                            wq_sb[:, dc, ts(pc, 128)],
                            xnT[:, dc, :],
                            start=(dc == 0),
                            stop=(dc == NDC - 1),
                        )
                    nc.any.tensor_copy(qT[:, pc, :], psq)
                    psk = ps_big.tile([128, SCH], F32, tag="big")
                    for dc in range(NDC):
                        nc.tensor.matmul(
                            psk,
                            wk_sb[:, dc, ts(pc, 128)],
                            xnT[:, dc, :],
                            start=(dc == 0),
                            stop=(dc == NDC - 1),
                        )
                    nc.any.tensor_copy(kT[pc][:, ds(j * SCH, SCH)], psk)
                for tt in range(NTT):
                    g = j * NTT + tt
                    psv = ps_big.tile([128, F_LOC], F32, tag="big")
                    for dc in range(NDC):
                        nc.tensor.matmul(
                            psv,
                            xnT[:, dc, ts(tt, 128)],
                            wv_sb[:, dc, :],
                            start=(dc == 0),
                            stop=(dc == NDC - 1),
                        )
                    v3 = vsb[g].rearrange("p (h c) -> p h c", h=H_LOC)
                    nc.any.tensor_copy(
                        v3[:, :, 0:64], psv.rearrange("p (h c) -> p h c", h=H_LOC)
                    )

                # ---------- attention for q-chunk j ----------
                aoT = slabs.tile([128, NPC, SCH], F32, tag="aoT")
                nkt = 4 * (j + 1)
                for h in range(H_LOC):
                    p, off = h // 2, (h % 2) * 64
                    po = ps_out.tile([65, SCH], F32, tag="out")
                    for kt in range(nkt):
                        psc = ps_sc.tile([128, SCH], F32, tag="sc")
                        nc.tensor.matmul(
                            psc,
                            kT[p][ds(off, 64), ts(kt, 128)],
                            qT[ds(off, 64), p, :],
                            start=True,
                            stop=True,
                        )
                        pt = ptp.tile([128, SCH], F32, tag="pt")
                        nc.scalar.activation(pt, psc, AF.Exp, scale=0.125)
                        delta = kt * 128 - j * SCH
                        if 0 <= delta <= 384:
                            nc.vector.tensor_mul(
                                pt, pt, mask_sb[:, ds(384 - delta, 512)]
                            )
                        nc.tensor.matmul(
                            po,
                            vsb[kt][:, ds(h * 65, 65)],
                            pt,
                            start=(kt == 0),
                            stop=(kt == nkt - 1),
                        )
                    den = dnp.tile([1, SCH], F32, tag="den")
                    nc.vector.reciprocal(den, po[64:65, :])
                    pbc = ps_sc.tile([64, SCH], F32, tag="sc")
                    nc.tensor.matmul(pbc, ones64, den, start=True, stop=True)
                    bc = ptp.tile([64, SCH], F32, tag="bc", bufs=2)
                    nc.any.tensor_copy(bc, pbc)
                    nc.vector.tensor_mul(aoT[ds(off, 64), p, :], po[0:64, :], bc)

                # ---------- output projection + residual ----------
                bounce_in = dram.tile([SCH, D], F32, tag="bin")
                for tt in range(NTT):
                    g = j * NTT + tt
                    xr = xp.tile([128, D], F32, tag="xr", bufs=2)
                    nc.sync.dma_start(out=xr[:], in_=x_ext[ds(g * 128, 128), :])
                    nc.vector.tensor_scalar_mul(xr, xr, 0.5)
                    for n in range(2):
                        psy = ps_big.tile([128, 512], F32, tag="big")
                        for pc in range(NPC):
                            nc.tensor.matmul(
                                psy,
                                aoT[:, pc, ts(tt, 128)],
                                wo_sb[:, pc, ds(n * 512, 512)],
                                start=(pc == 0),
                                stop=(pc == NPC - 1),
                            )
                        nc.vector.tensor_add(
                            xr[:, ds(n * 512, 512)], xr[:, ds(n * 512, 512)], psy
                        )
                    nc.sync.dma_start(out=bounce_in[ds(tt * 128, 128), :], in_=xr[:])

                # ---------- pairwise ReduceScatter of this chunk ----------
                bounce_out = dram.tile([SCH // 2, D], F32, tag="bout")
                nc.gpsimd.collective_compute(
                    "ReduceScatter",
                    mybir.AluOpType.add,
                    replica_groups=RG,
                    ins=[bounce_in.opt()],
                    outs=[bounce_out.opt()],
                )
                nc.sync.dma_start(
                    out=out_ext[ds(j * (SCH // 2), SCH // 2), :], in_=bounce_out[:]
                )

    nc.compile()
    return nc


_CACHE: dict = {}


def _get_nc():
    if "nc" not in _CACHE:
        _CACHE["nc"] = build()
    return _CACHE["nc"]


def _make_mask() -> np.ndarray:
    k = np.arange(128)[:, None]
    u = np.arange(896)[None, :]
    return (k <= u - 384).astype(np.float32)


def make_in_maps(x, Wq, bq, Wk, bk, Wv, bv, Wo, bo, gamma, beta):
    x = np.asarray(x, dtype=np.float32)
    for name, b in (("bq", bq), ("bk", bk), ("bv", bv), ("bo", bo), ("beta", beta)):
        if np.abs(np.asarray(b)).max() > 1e-12:
            raise NotImplementedError(f"nonzero {name} not supported by this kernel")
    g = np.asarray(gamma, dtype=np.float32)[:, None]
    wq = np.ascontiguousarray(g * np.asarray(Wq, dtype=np.float32))
    wk = np.ascontiguousarray(g * np.asarray(Wk, dtype=np.float32))
    wv = np.ascontiguousarray(g * np.asarray(Wv, dtype=np.float32))
    wo = np.ascontiguousarray(np.asarray(Wo, dtype=np.float32))
    mask = _make_mask()
    in_maps = []
    for r in range(8):
        b, hg = r // 2, r % 2
        cs = slice(hg * F_LOC, (hg + 1) * F_LOC)
        in_maps.append(
            {
                "x": np.ascontiguousarray(x[b]),
                "wq": np.ascontiguousarray(wq[:, cs]),
                "wk": np.ascontiguousarray(wk[:, cs]),
                "wv": np.ascontiguousarray(wv[:, cs]),
                "wo": np.ascontiguousarray(wo[cs, :]),
                "mask": mask,
            }
        )
    return in_maps


def assemble(results) -> np.ndarray:
    out = np.empty((B, S, D), dtype=np.float32)
    half = SCH // 2
    for p in range(B):
        lo = results[2 * p]["out"]
        hi = results[2 * p + 1]["out"]
        for j in range(NCH):
            out[p, j * SCH : j * SCH + half] = lo[j * half : (j + 1) * half]
            out[p, j * SCH + half : (j + 1) * SCH] = hi[j * half : (j + 1) * half]
    return out


def kernel(**inputs) -> np.ndarray:
    nc = _get_nc()
    in_maps = make_in_maps(**inputs)
    res = run_bass_kernel_spmd(nc, in_maps, core_ids=list(range(8)))
    return assemble(res.results)


if __name__ == "__main__":
    rng = np.random.default_rng(0)
    demo = {
        "x": rng.standard_normal((B, S, D), dtype=np.float32),
        "Wq": rng.standard_normal((D, H * DK), dtype=np.float32) / 32,
        "bq": np.zeros(H * DK, np.float32),
        "Wk": rng.standard_normal((D, H * DK), dtype=np.float32) / 32,
        "bk": np.zeros(H * DK, np.float32),
        "Wv": rng.standard_normal((D, H * DK), dtype=np.float32) / 32,
        "bv": np.zeros(H * DK, np.float32),
        "Wo": rng.standard_normal((H * DK, D), dtype=np.float32) / 32,
        "bo": np.zeros(D, np.float32),
        "gamma": np.ones(D, np.float32),
        "beta": np.zeros(D, np.float32),
    }
    out = kernel(**demo)
    print("out", out.shape, out.dtype, np.abs(out).mean())


# revision 4
# speedup vs baseline: 2.1942x; 2.1942x over previous
"""Distributed Trainium2 kernel for pre-LN causal multi-head attention.

Problem: out = x + Wo-proj(causal-MHA(LN(x))) with B=4, S=2048, D=1024,
H=16 heads, d_k=d_v=64, fp32 inputs/outputs.

Sharding over 8 NeuronCores (per the TP/DP hint):
  core r -> batch b = r//2, head group g = r%2 (heads 8g..8g+7).
  Wq/Wk/Wv column-sliced per head group, Wo row-sliced; the two cores of a
  pair {2p, 2p+1} each compute a partial output projection for batch p and
  a pairwise ReduceScatter (+ pre-added x/2 residual on each core) yields
  final output rows split across the pair.

Single-core layout strategy (everything transposed once, then no more
transposes):
  - LN stats in natural layout (bn_stats), gamma/beta folded into the
    projection weights host-side, so the device only standardizes.
  - matmul operands in bf16 (fp32 matmuls cost 2 PE passes on trn2; bf16
    costs 1), all accumulation in fp32 PSUM; the residual path stays fp32
    so the output error is only on the attention delta (~1e-3 rel).
  - xn^T via PE transposes; Q/K projections then produce q^T/k^T
    ([feature, token], features = head pairs stacked 64+64 on partitions)
    and V is produced in natural [token, feature] layout directly.
  - scores are computed TRANSPOSED: s^T[k, q] = k^T(stationary) x q^T(moving),
    two heads concurrently via PE row groups (base partitions 0 / 64).
  - softmax over k = partition axis of s^T: exp on ACT (no max subtraction
    needed: |scores/8| < ~3 by construction), denominator via a ones column
    appended to V in the P^T @ V matmul, causal mask via a precomputed
    sliding band mask multiply on the diagonal tiles only (plus skipping
    fully-masked tiles). 1/denominator is partition-broadcast on GpSimd.
  - attn^T [d_v, q] feeds the output projection as stationary operand,
    producing y in natural [token, d_model] layout; +x/2 residual, then
    chunked (512-token) pairwise ReduceScatter overlapped with compute.
"""

import ml_dtypes
import numpy as np

import concourse.bass as bass
import concourse.tile as tile
from concourse import bacc, mybir
from concourse.bass import ds, ts
from concourse.bass_utils import run_bass_kernel_spmd
from concourse.masks import make_identity

F32 = mybir.dt.float32
BF16 = mybir.dt.bfloat16
AF = mybir.ActivationFunctionType

B = 4
S = 2048
D = 1024
H = 16
DK = 64
H_LOC = 8            # heads per core
F_LOC = H_LOC * DK   # 512 local features
SCH = 512            # token chunk (pipeline + RS granularity)
NCH = S // SCH       # 4 chunks
NTT = SCH // 128     # 4 token tiles per chunk
NDC = D // 128       # 8 d_model chunks
NPC = F_LOC // 128   # 4 feature pair-chunks (2 heads each)
NKT = S // 128       # 16 key tiles
EPS = 1e-5
RG = [[0, 1], [2, 3], [4, 5], [6, 7]]


def build(n_chunks: int = NCH):
    """Build the SPMD graph (identical on all 8 cores)."""
    nc = bacc.Bacc("TRN2", target_bir_lowering=False, debug=False, num_devices=8)

    s_loc = n_chunks * SCH
    x_ext = nc.dram_tensor("x", [s_loc, D], F32, kind="ExternalInput").ap()
    wq_ext = nc.dram_tensor("wq", [D, F_LOC], BF16, kind="ExternalInput").ap()
    wk_ext = nc.dram_tensor("wk", [D, F_LOC], BF16, kind="ExternalInput").ap()
    wv_ext = nc.dram_tensor("wv", [D, F_LOC], BF16, kind="ExternalInput").ap()
    wo_ext = nc.dram_tensor("wo", [F_LOC, D], BF16, kind="ExternalInput").ap()
    mask_ext = nc.dram_tensor("mask", [128, 896], BF16, kind="ExternalInput").ap()
    out_ext = nc.dram_tensor("out", [s_loc // 2, D], F32, kind="ExternalOutput").ap()

    with tile.TileContext(nc) as tc:
        with (
            tc.tile_pool(name="persist", bufs=1) as persist,
            tc.tile_pool(name="slabs", bufs=2) as slabs,
            tc.tile_pool(name="xp", bufs=3) as xp,
            tc.tile_pool(name="ptp", bufs=6) as ptp,
            tc.tile_pool(name="dnp", bufs=2) as dnp,
            tc.tile_pool(name="stp", bufs=3) as stp,
            tc.tile_pool(name="ps_tr", bufs=2, space="PSUM") as ps_tr,
            tc.tile_pool(name="ps_big", bufs=2, space="PSUM") as ps_big,
            tc.tile_pool(name="ps_sc", bufs=2, space="PSUM") as ps_sc,
            tc.tile_pool(name="ps_out", bufs=2, space="PSUM") as ps_out,
            tc.tile_pool(name="dram", bufs=2, space="DRAM") as dram,
        ):
            # ---- persistent tiles ----
            wq_sb = persist.tile([128, NDC, F_LOC], BF16)
            wk_sb = persist.tile([128, NDC, F_LOC], BF16)
            wv_sb = persist.tile([128, NDC, F_LOC], BF16)
            wo_sb = persist.tile([128, NPC, D], BF16)
            for dc in range(NDC):
                nc.sync.dma_start(out=wq_sb[:, dc, :], in_=wq_ext[ds(dc * 128, 128), :])
                nc.sync.dma_start(out=wk_sb[:, dc, :], in_=wk_ext[ds(dc * 128, 128), :])
                nc.sync.dma_start(out=wv_sb[:, dc, :], in_=wv_ext[ds(dc * 128, 128), :])
            for pc in range(NPC):
                nc.sync.dma_start(out=wo_sb[:, pc, :], in_=wo_ext[ds(pc * 128, 128), :])

            mask_sb = persist.tile([128, 896], BF16)
            nc.sync.dma_start(out=mask_sb[:], in_=mask_ext[:])
            ident = persist.tile([128, 128], BF16)
            make_identity(nc, ident)
            epsb = persist.tile([128, 1], F32)
            nc.vector.memset(epsb, EPS)

            # k^T per head pair: [128 (= 2x64 head dims), S]
            kT = [persist.tile([128, S], BF16, name=f"kT{p}") for p in range(NPC)]
            # v (+ ones col per head) per key tile: [128 tokens, 8*(64+1)]
            vsb = [persist.tile([128, H_LOC * 65], BF16, name=f"v{t}") for t in range(NKT)]
            for t in range(n_chunks * NTT):
                v3 = vsb[t].rearrange("p (h c) -> p h c", h=H_LOC)
                nc.vector.memset(v3[:, :, 64:65], 1.0)

            for j in range(n_chunks):
                # ---------- LayerNorm + transpose ----------
                xnT = slabs.tile([128, NDC, SCH], BF16, tag="xnT")
                for tt in range(NTT):
                    g = j * NTT + tt
                    x_t = xp.tile([128, D], F32, tag="x_t")
                    nc.sync.dma_start(out=x_t[:], in_=x_ext[ds(g * 128, 128), :])
                    st6 = stp.tile([128, 2, 6], F32)
                    nc.vector.bn_stats(st6[:, 0, :], x_t[:, 0:512])
                    nc.vector.bn_stats(st6[:, 1, :], x_t[:, 512:1024])
                    mv = stp.tile([128, 2], F32)
                    nc.vector.bn_aggr(mv, st6)
                    rstd = stp.tile([128, 1], F32)
                    nc.scalar.activation(rstd, mv[:, 1:2], AF.Sqrt, bias=epsb)
                    nc.vector.reciprocal(rstd, rstd)
                    xs = xp.tile([128, D], BF16, tag="xs")
                    nc.vector.tensor_scalar(
                        out=xs[:],
                        in0=x_t[:],
                        scalar1=mv[:, 0:1],
                        scalar2=rstd,
                        op0=mybir.AluOpType.subtract,
                        op1=mybir.AluOpType.mult,
                    )
                    for half in range(2):
                        ptr = ps_tr.tile([128, 512], BF16, tag="tr")
                        for q in range(4):
                            dc = half * 4 + q
                            nc.tensor.transpose(
                                ptr[:, ts(q, 128)], xs[:, ts(dc, 128)], ident
                            )
                        nc.any.tensor_copy(
                            xnT[:, ds(half * 4, 4), ts(tt, 128)],
                            ptr.rearrange("p (c n) -> p c n", c=4),
                        )

                # ---------- Q/K/V projections for this chunk ----------
                qT = slabs.tile([128, NPC, SCH], BF16, tag="qT")
                for pc in range(NPC):
                    psq = ps_big.tile([128, SCH], F32, tag="big")
                    for dc in range(NDC):
                        nc.tensor.matmul(
                            psq,
                            wq_sb[:, dc, ts(pc, 128)],
                            xnT[:, dc, :],
                            start=(dc == 0),
                            stop=(dc == NDC - 1),
                        )
                    nc.any.tensor_copy(qT[:, pc, :], psq)
                    psk = ps_big.tile([128, SCH], F32, tag="big")
                    for dc in range(NDC):
                        nc.tensor.matmul(
                            psk,
                            wk_sb[:, dc, ts(pc, 128)],
                            xnT[:, dc, :],
                            start=(dc == 0),
                            stop=(dc == NDC - 1),
                        )
                    nc.any.tensor_copy(kT[pc][:, ds(j * SCH, SCH)], psk)
                for tt in range(NTT):
                    g = j * NTT + tt
                    psv = ps_big.tile([128, F_LOC], F32, tag="big")
                    for dc in range(NDC):
                        nc.tensor.matmul(
                            psv,
                            xnT[:, dc, ts(tt, 128)],
                            wv_sb[:, dc, :],
                            start=(dc == 0),
                            stop=(dc == NDC - 1),
                        )
                    v3 = vsb[g].rearrange("p (h c) -> p h c", h=H_LOC)
                    nc.any.tensor_copy(
                        v3[:, :, 0:64], psv.rearrange("p (h c) -> p h c", h=H_LOC)
                    )

                # ---------- attention for q-chunk j ----------
                aoT = slabs.tile([128, NPC, SCH], BF16, tag="aoT")
                nkt = 4 * (j + 1)
                for h in range(H_LOC):
                    p, off = h // 2, (h % 2) * 64
                    po = ps_out.tile([65, SCH], F32, tag="out")
                    for kt in range(nkt):
                        psc = ps_sc.tile([128, SCH], F32, tag="sc")
                        nc.tensor.matmul(
                            psc,
                            kT[p][ds(off, 64), ts(kt, 128)],
                            qT[ds(off, 64), p, :],
                            start=True,
                            stop=True,
                        )
                        pt = ptp.tile([128, SCH], BF16, tag="pt")
                        nc.scalar.activation(pt, psc, AF.Exp, scale=0.125)
                        delta = kt * 128 - j * SCH
                        if 0 <= delta <= 384:
                            nc.vector.tensor_mul(
                                pt, pt, mask_sb[:, ds(384 - delta, 512)]
                            )
                        nc.tensor.matmul(
                            po,
                            vsb[kt][:, ds(h * 65, 65)],
                            pt,
                            start=(kt == 0),
                            stop=(kt == nkt - 1),
                        )
                    den = dnp.tile([1, SCH], F32, tag="den")
                    nc.vector.reciprocal(den, po[64:65, :])
                    bc = ptp.tile([64, SCH], F32, tag="bc", bufs=2)
                    nc.gpsimd.partition_broadcast(bc, den)
                    nc.vector.tensor_mul(aoT[ds(off, 64), p, :], po[0:64, :], bc)

                # ---------- output projection + residual ----------
                bounce_in = dram.tile([SCH, D], F32, tag="bin")
                for tt in range(NTT):
                    g = j * NTT + tt
                    xr = xp.tile([128, D], F32, tag="xr", bufs=2)
                    nc.sync.dma_start(out=xr[:], in_=x_ext[ds(g * 128, 128), :])
                    nc.vector.tensor_scalar_mul(xr, xr, 0.5)
                    for n in range(2):
                        psy = ps_big.tile([128, 512], F32, tag="big")
                        for pc in range(NPC):
                            nc.tensor.matmul(
                                psy,
                                aoT[:, pc, ts(tt, 128)],
                                wo_sb[:, pc, ds(n * 512, 512)],
                                start=(pc == 0),
                                stop=(pc == NPC - 1),
                            )
                        nc.vector.tensor_add(
                            xr[:, ds(n * 512, 512)], xr[:, ds(n * 512, 512)], psy
                        )
                    nc.sync.dma_start(out=bounce_in[ds(tt * 128, 128), :], in_=xr[:])

                # ---------- pairwise ReduceScatter of this chunk ----------
                bounce_out = dram.tile([SCH // 2, D], F32, tag="bout")
                nc.gpsimd.collective_compute(
                    "ReduceScatter",
                    mybir.AluOpType.add,
                    replica_groups=RG,
                    ins=[bounce_in.opt()],
                    outs=[bounce_out.opt()],
                )
                nc.sync.dma_start(
                    out=out_ext[ds(j * (SCH // 2), SCH // 2), :], in_=bounce_out[:]
                )

    nc.compile()
    return nc


_CACHE: dict = {}


def _get_nc():
    if "nc" not in _CACHE:
        _CACHE["nc"] = build()
    return _CACHE["nc"]


def _make_mask() -> np.ndarray:
    k = np.arange(128)[:, None]
    u = np.arange(896)[None, :]
    return (k <= u - 384).astype(ml_dtypes.bfloat16)


def make_in_maps(x, Wq, bq, Wk, bk, Wv, bv, Wo, bo, gamma, beta):
    x = np.asarray(x, dtype=np.float32)
    for name, b in (("bq", bq), ("bk", bk), ("bv", bv), ("bo", bo), ("beta", beta)):
        if np.abs(np.asarray(b)).max() > 1e-12:
            raise NotImplementedError(f"nonzero {name} not supported by this kernel")
    g = np.asarray(gamma, dtype=np.float32)[:, None]
    wq = (g * np.asarray(Wq, dtype=np.float32)).astype(ml_dtypes.bfloat16)
    wk = (g * np.asarray(Wk, dtype=np.float32)).astype(ml_dtypes.bfloat16)
    wv = (g * np.asarray(Wv, dtype=np.float32)).astype(ml_dtypes.bfloat16)
    wo = np.asarray(Wo, dtype=np.float32).astype(ml_dtypes.bfloat16)
    mask = _make_mask()
    in_maps = []
    for r in range(8):
        b, hg = r // 2, r % 2
        cs = slice(hg * F_LOC, (hg + 1) * F_LOC)
        in_maps.append(
            {
                "x": np.ascontiguousarray(x[b]),
                "wq": np.ascontiguousarray(wq[:, cs]),
                "wk": np.ascontiguousarray(wk[:, cs]),
                "wv": np.ascontiguousarray(wv[:, cs]),
                "wo": np.ascontiguousarray(wo[cs, :]),
                "mask": mask,
            }
        )
    return in_maps


def assemble(results) -> np.ndarray:
    out = np.empty((B, S, D), dtype=np.float32)
    half = SCH // 2
    for p in range(B):
        lo = results[2 * p]["out"]
        hi = results[2 * p + 1]["out"]
        for j in range(NCH):
            out[p, j * SCH : j * SCH + half] = lo[j * half : (j + 1) * half]
            out[p, j * SCH + half : (j + 1) * SCH] = hi[j * half : (j + 1) * half]
    return out


def kernel(**inputs) -> np.ndarray:
    nc = _get_nc()
    in_maps = make_in_maps(**inputs)
    res = run_bass_kernel_spmd(nc, in_maps, core_ids=list(range(8)))
    return assemble(res.results)


if __name__ == "__main__":
    rng = np.random.default_rng(0)
    demo = {
        "x": rng.standard_normal((B, S, D), dtype=np.float32),
        "Wq": rng.standard_normal((D, H * DK), dtype=np.float32) / 32,
        "bq": np.zeros(H * DK, np.float32),
        "Wk": rng.standard_normal((D, H * DK), dtype=np.float32) / 32,
        "bk": np.zeros(H * DK, np.float32),
        "Wv": rng.standard_normal((D, H * DK), dtype=np.float32) / 32,
        "bv": np.zeros(H * DK, np.float32),
        "Wo": rng.standard_normal((H * DK, D), dtype=np.float32) / 32,
        "bo": np.zeros(D, np.float32),
        "gamma": np.ones(D, np.float32),
        "beta": np.zeros(D, np.float32),
    }
    out = kernel(**demo)
    print("out", out.shape, out.dtype, np.abs(out).mean())
